# revision 1
# baseline (speedup 1.0000x reference)
"""Trainium2 Bass kernel for nn_Encoder_GCN_30013231464613.

GCN message-passing encoder (3 GCNConv layers + per-layer global mean pool)
on 8 NeuronCores.  Nodes are sharded contiguously across cores (edge-cut by
destination).  Per layer: h = x @ W for local nodes, AllGather h into every
core's HBM, then each core aggregates incoming messages for its dst nodes:
batched src-sorted dma_gather of h[src] rows, segment-sum on the PE via
host-precomputed selection matrices (GCN norm folded in), fused bias+ReLU on
the scalar engine in transposed orientation.  Mean-pool and the final
transpose run on the host.
"""
import sys
sys.path.insert(0, '/opt/trn_rl_repo')
import math
import numpy as np

import concourse.bass as bass
import concourse.bacc as bacc
import concourse.mybir as mybir
import concourse.tile as tile

F32 = mybir.dt.float32
I16 = mybir.dt.int16

CPI = 32          # chunks per gather / S instruction
D = 128


class Cfg:
    def __init__(self, n_nodes, n_edges, n_graphs, n_cores=8):
        assert n_nodes % n_cores == 0
        self.N = n_nodes
        self.E = n_edges
        self.NG = n_graphs
        self.NC = n_cores
        self.NPC = n_nodes // n_cores          # nodes per core
        self.NB = math.ceil(self.NPC / 128)    # dst blocks per core
        self.HALF = n_nodes // 2               # table split point (int16 idx)
        assert self.HALF <= 32768


def host_prep(cfg, x, edge_index, W0, b0, W1, b1, W2, b2):
    """Build per-core device inputs + the (shared) chunk schedule."""
    N, NC, NPC, NB, HALF = cfg.N, cfg.NC, cfg.NPC, cfg.NB, cfg.HALF
    src = np.concatenate([np.asarray(edge_index[0]),
                          np.arange(N, dtype=np.int64)])
    dst = np.concatenate([np.asarray(edge_index[1]),
                          np.arange(N, dtype=np.int64)])
    deg = np.bincount(dst, minlength=N).astype(np.float64)
    dinv = 1.0 / np.sqrt(deg)                  # deg >= 1 (self loops)
    norm = (dinv[src] * dinv[dst]).astype(np.float32)

    core = dst // NPC
    side = (src >= HALF).astype(np.int64)
    blk = (dst % NPC) // 128
    dloc = (dst % NPC) % 128
    src_adj = np.where(side == 1, src - HALF, src)

    order = np.lexsort((src_adj, side, blk, core))
    so_src = src_adj[order].astype(np.int64)
    so_dloc = dloc[order]
    so_norm = norm[order]
    so_core, so_blk, so_side = core[order], blk[order], side[order]

    counts = np.zeros((NC, NB, 2), np.int64)
    np.add.at(counts, (so_core, so_blk, so_side), 1)
    nch = np.ceil(counts / 128.0).astype(np.int64).max(axis=0)  # [NB, 2]

    # global chunk stream: blocks ascending, side 0 then 1
    # chunk meta: (blk, side, first, last, side_instr, side_slot, sgrp, sslot)
    chunk_meta = []
    scount = [0, 0]
    chunk_k = {}
    for b in range(NB):
        tot = int(nch[b, 0] + nch[b, 1])
        assert tot >= 1
        k = 0
        for s in (0, 1):
            for j in range(int(nch[b, s])):
                g = len(chunk_meta)
                chunk_meta.append((b, s, k == 0, k == tot - 1,
                                   scount[s] // CPI, scount[s] % CPI,
                                   g // CPI, g % CPI))
                chunk_k[(b, s, j)] = g
                scount[s] += 1
                k += 1
    NCH = len(chunk_meta)
    n_side = [scount[0], scount[1]]
    NI = [max(1, math.ceil(n_side[0] / CPI)), max(1, math.ceil(n_side[1] / CPI))]
    NSI = math.ceil(NCH / CPI)
    side_instr_nch = [[0] * NI[0], [0] * NI[1]]
    for (b, s, fi, la, si, sslot, gi, gslot) in chunk_meta:
        side_instr_nch[s][si] += 1

    inputs = []
    for c in range(NC):
        Sarr = np.zeros((NSI, 128, CPI * 128), np.float32)
        toks = [np.zeros((NI[0] * CPI * 128,), np.int64),
                np.zeros((NI[1] * CPI * 128,), np.int64)]
        msk = so_core == c
        csrc, cdloc, cnorm = so_src[msk], so_dloc[msk], so_norm[msk]
        cblk, cside = so_blk[msk], so_side[msk]
        key = cblk * 2 + cside
        L = len(key)
        if L:
            starts = np.flatnonzero(np.r_[True, key[1:] != key[:-1]])
            run_len = np.bincount(key, minlength=NB * 2)
            run_start = np.zeros(NB * 2, np.int64)
            run_start[key[starts]] = starts
        else:
            run_len = np.zeros(NB * 2, np.int64)
            run_start = np.zeros(NB * 2, np.int64)
        for b in range(NB):
            for s in (0, 1):
                ln = int(run_len[b * 2 + s])
                st = int(run_start[b * 2 + s])
                for j in range(int(nch[b, s])):
                    lo, hi = min(j * 128, ln), min(j * 128 + 128, ln)
                    n = hi - lo
                    if n <= 0:
                        continue
                    g = chunk_k[(b, s, j)]
                    _, _, _, _, si, sslot, gi, gslot = chunk_meta[g]
                    sl = slice(st + lo, st + hi)
                    rows = np.arange(n)
                    Sarr[gi, rows, gslot * 128 + cdloc[sl]] = cnorm[sl]
                    toks[s][(si * CPI + sslot) * 128 + rows] = csrc[sl]
        idxs = []
        for s in (0, 1):
            t = toks[s].reshape(NI[s], CPI * 128).astype(np.int16)
            w = t.reshape(NI[s], CPI * 8, 16).transpose(0, 2, 1)  # [NI,16,NF]
            idxs.append(np.ascontiguousarray(np.tile(w, (1, 8, 1))))
        xT0 = np.ascontiguousarray(
            np.asarray(x)[c * NPC:(c + 1) * NPC].T).astype(np.float32)
        inputs.append({
            "xT0": xT0, "Sarr": Sarr, "idx0": idxs[0], "idx1": idxs[1],
            "W0": np.asarray(W0, np.float32), "W1": np.asarray(W1, np.float32),
            "W2": np.asarray(W2, np.float32),
            "b0": np.asarray(b0, np.float32).reshape(-1, 1),
            "b1": np.asarray(b1, np.float32).reshape(-1, 1),
            "b2": np.asarray(b2, np.float32).reshape(-1, 1),
        })
    meta = dict(sched=chunk_meta, NCH=NCH, NI=NI, NSI=NSI,
                side_instr_nch=side_instr_nch)
    return inputs, meta


def build_program(cfg, meta):
    NPC, NB, HALF, N = cfg.NPC, cfg.NB, cfg.HALF, cfg.N
    NI, NSI = meta["NI"], meta["NSI"]
    chunk_meta = meta["sched"]
    side_instr_nch = meta["side_instr_nch"]
    nc = bacc.Bacc("TRN2", target_bir_lowering=False, debug=False,
                   num_devices=cfg.NC)
    xT0_in = nc.dram_tensor("xT0", [128, NPC], F32, kind="ExternalInput")
    S_in = nc.dram_tensor("Sarr", [NSI, 128, CPI * 128], F32,
                          kind="ExternalInput")
    idx_in = [nc.dram_tensor("idx0", [NI[0], 128, CPI * 8], I16,
                             kind="ExternalInput"),
              nc.dram_tensor("idx1", [NI[1], 128, CPI * 8], I16,
                             kind="ExternalInput")]
    W_in = [nc.dram_tensor(f"W{l}", [128, 128], F32, kind="ExternalInput")
            for l in range(3)]
    b_in = [nc.dram_tensor(f"b{l}", [128, 1], F32, kind="ExternalInput")
            for l in range(3)]
    xs_out = nc.dram_tensor("xs", [3 * 128, NPC], F32, kind="ExternalOutput")
    groups = [list(range(cfg.NC))]

    with tile.TileContext(nc) as tc:
        with (
            tc.tile_pool(name="dram", bufs=1, space="DRAM") as dpool,
            tc.tile_pool(name="const", bufs=1) as cpool,
            tc.tile_pool(name="x0", bufs=3) as x0pool,
            tc.tile_pool(name="g", bufs=3) as gpool,
            tc.tile_pool(name="sp", bufs=3) as spool,
            tc.tile_pool(name="ip", bufs=3) as ipool,
            tc.tile_pool(name="xt", bufs=3) as xtpool,
            tc.tile_pool(name="hsb", bufs=3) as hpool,
            tc.tile_pool(name="psA", bufs=2, space="PSUM") as psA,
            tc.tile_pool(name="psB", bufs=2, space="PSUM") as psB,
        ):
            h_stage = dpool.tile([NPC, D], F32, name="h_stage")
            h_fulls = [dpool.tile([N, D], F32, name=f"h_full{l}",
                                  addr_space="Shared") for l in range(3)]
            Wt, bt = [], []
            for l in range(3):
                w = cpool.tile([128, 128], F32, tag=f"W{l}", name=f"Wt{l}")
                nc.sync.dma_start(out=w[:], in_=W_in[l][:])
                Wt.append(w)
                b = cpool.tile([128, 1], F32, tag=f"b{l}", name=f"bt{l}")
                nc.sync.dma_start(out=b[:], in_=b_in[l][:])
                bt.append(b)

            def last_w(b):
                return min(128, NPC - b * 128)

            # ---- phase 0: h0 = x @ W0 for local nodes ----
            for b in range(NB):
                w = last_w(b)
                xt = x0pool.tile([128, 128], F32, tag="x0", name="x0t")
                nc.sync.dma_start(out=xt[:, :w],
                                  in_=xT0_in[:, b * 128:b * 128 + w])
                hp = psB.tile([128, 128], F32, tag="hps", name="hps0")
                nc.tensor.matmul(out=hp[:w, :], lhsT=xt[:, :w], rhs=Wt[0][:],
                                 start=True, stop=True)
                hs = hpool.tile([128, 128], F32, tag="h", name="hs0")
                nc.vector.tensor_copy(out=hs[:w, :], in_=hp[:w, :])
                nc.sync.dma_start(out=h_stage[b * 128:b * 128 + w, :],
                                  in_=hs[:w, :])
            nc.gpsimd.collective_compute(
                "AllGather", mybir.AluOpType.bypass, replica_groups=groups,
                ins=[h_stage[:]], outs=[h_fulls[0][:]])

            # ---- layers ----
            for l in range(3):
                g_tiles = [{}, {}]
                s_tiles = {}
                xps = None
                for (b, s, first, last_c, si, sslot, gi, gslot) in chunk_meta:
                    if si not in g_tiles[s]:
                        nchn = side_instr_nch[s][si]
                        it = ipool.tile([128, CPI * 8], I16, tag="idx",
                                        name="idxt")
                        nc.sync.dma_start(out=it[:, :nchn * 8],
                                          in_=idx_in[s][si, :, :nchn * 8])
                        gt = gpool.tile([128, CPI * 128], F32, tag="g",
                                        name="gt")
                        out_ap = gt[:, :nchn * 128].rearrange(
                            "p (k d) -> p k d", k=nchn)
                        half_ap = h_fulls[l][0:HALF, :] if s == 0 \
                            else h_fulls[l][HALF:N, :]
                        nc.gpsimd.dma_gather(
                            out_ap=out_ap, in_ap=half_ap,
                            idxs_ap=it[:, :nchn * 8],
                            num_idxs=nchn * 128, num_idxs_reg=nchn * 128,
                            elem_size=D, single_packet=False)
                        g_tiles[s][si] = gt
                    if gi not in s_tiles:
                        st = spool.tile([128, CPI * 128], F32, tag="S",
                                        name="st")
                        nc.sync.dma_start(out=st[:], in_=S_in[gi])
                        s_tiles[gi] = st
                    if first:
                        xps = psA.tile([128, 128], F32, tag="xps", name="xps")
                    gt = g_tiles[s][si]
                    st = s_tiles[gi]
                    nc.tensor.matmul(
                        out=xps[:],
                        lhsT=gt[:, sslot * 128:(sslot + 1) * 128],
                        rhs=st[:, gslot * 128:(gslot + 1) * 128],
                        start=first, stop=last_c)
                    if last_c:
                        w = last_w(b)
                        xt = xtpool.tile([128, 128], F32, tag="xt", name="xtt")
                        nc.scalar.activation(
                            out=xt[:], in_=xps[:],
                            func=mybir.ActivationFunctionType.Relu,
                            bias=bt[l][:, :1], scale=1.0)
                        nc.sync.dma_start(
                            out=xs_out[l * 128:(l + 1) * 128,
                                       b * 128:b * 128 + w],
                            in_=xt[:, :w])
                        if l < 2:
                            hp = psB.tile([128, 128], F32, tag="hps",
                                          name="hps")
                            nc.tensor.matmul(out=hp[:w, :], lhsT=xt[:, :w],
                                             rhs=Wt[l + 1][:],
                                             start=True, stop=True)
                            hs = hpool.tile([128, 128], F32, tag="h",
                                            name="hs")
                            nc.vector.tensor_copy(out=hs[:w, :],
                                                  in_=hp[:w, :])
                            nc.sync.dma_start(
                                out=h_stage[b * 128:b * 128 + w, :],
                                in_=hs[:w, :])
                if l < 2:
                    nc.gpsimd.collective_compute(
                        "AllGather", mybir.AluOpType.bypass,
                        replica_groups=groups,
                        ins=[h_stage[:]], outs=[h_fulls[l + 1][:]])
    nc.compile()
    return nc


def postprocess(cfg, batch, core_outs):
    """core_outs: list of xs [384, NPC] per core -> (xpool, xs_full)."""
    NPC = cfg.NPC
    batch = np.asarray(batch)
    xs = np.empty((cfg.N, 384), np.float32)
    for c in range(cfg.NC):
        xs[c * NPC:(c + 1) * NPC, :] = np.asarray(core_outs[c]).T
    counts = np.bincount(batch, minlength=cfg.NG).astype(np.float32)
    counts = np.maximum(counts, 1.0)
    bounds = np.searchsorted(batch, np.arange(cfg.NG + 1))
    ext = np.vstack([xs, np.zeros((1, 384), np.float32)])
    xpool = np.add.reduceat(ext, bounds[:-1], axis=0)[:cfg.NG]
    empty = (bounds[1:] - bounds[:-1]) == 0
    xpool[empty] = 0.0
    xpool = (xpool / counts[:, None]).astype(np.float32)
    return xpool, xs


_CFG = Cfg(50000, 800000, 128)


def kernel(x, edge_index, batch, W0, b0, W1, b1, W2, b2):
    from concourse.bass_utils import run_bass_kernel_spmd
    x = np.asarray(x)
    edge_index = np.asarray(edge_index)
    batch = np.asarray(batch)
    in_maps, meta = host_prep(_CFG, x, edge_index, W0, b0, W1, b1, W2, b2)
    nc = build_program(_CFG, meta)
    res = run_bass_kernel_spmd(nc, in_maps, core_ids=list(range(_CFG.NC)))
    core_xs = [res.results[c]["xs"] for c in range(_CFG.NC)]
    xpool, xs = postprocess(_CFG, batch, core_xs)
    return xpool, xs
